# revision 16
# baseline (speedup 1.0000x reference)
"""DiversityAttention on 8 TRN2 NeuronCores (Bass/Tile), v3.

Sharding: data-parallel over batch (B=2) x tensor-parallel over heads
(16 heads -> 4 groups of 4). core = (b, g), b = core // 4, g = core % 4.
Each core computes full attention for its 4 heads over its batch and a
partial out-projection [S, HIDDEN]; the host sums the 4 partials per
batch and adds bo.

Keys-on-partitions orientation, all-bf16 matmuls except the fp8
DoubleRow similarity:
  qT = (Wq/8 @ x^T + bq/8)  [64h, S] bf16    (1/sqrt(dh) folded on host)
  kT = (Wk   @ x^T + bk)    [64h, S] bf16
  V  = x @ WvT + bv directly in [keys, dh] layout, bf16 + ones column
  xh8 = fp8e4(x^T * 64*sqrt(gamma)/||x||)  (DoubleRow-interleaved)
  per (qb, kt):
     sim_psum = xh8^T xh8 (fp8 DoubleRow) = 4096*gamma*sim
     E = exp(-sim_psum/4096) (ACT, scale folded)          [128, QB] bf16
     scores_psum = kT^T qT per head pair (row-packed 64-contraction)
     es = exp(scores_psum) (ACT reads PSUM)               bf16
     pt = es * E (DVE bf16 2x)
     ctx_psum[65, QB] += [V|1]^T pt  (per head, ones row = softmax sums)
  sim/scores/out-proj PSUM tiles rotate through two 2-bank tags so the
  exp of one tile overlaps matmuls into the other; ctx holds 4 banks.
  at qb end: copy sums -> recip_approx_fast -> gpsimd broadcast -> DVE
  mul gives ctxT2 bf16; qb's out-projection is spread over kt=1..4 of
  the next qb so the PE never idles at block boundaries.
"""

import math
import os
import sys

import numpy as np

for _p in ("/opt/trn_rl_repo",):
    if _p not in sys.path and os.path.isdir(_p):
        sys.path.insert(0, _p)

os.environ.setdefault("MYCRO_LOCAL_CACHE", "1")

import concourse.bass as bass
import concourse.tile as tile
from concourse import bacc, mybir
from concourse.bass_utils import run_bass_kernel_spmd


def _install_ntff_hook():
    """Provide antenv.axon_hooks (NTFF profiling registry) if the image
    lacks it, mirroring trn_agent_boot's ctypes hook. No-op on failure."""
    try:
        import antenv.axon_hooks  # noqa: F401
        return
    except ImportError:
        pass
    try:
        import contextlib
        import ctypes
        import types

        so_path = "/opt/axon/libaxon_pjrt.so"
        if not os.path.exists(so_path):
            return
        lib = ctypes.CDLL(so_path)
        if not hasattr(lib, "axon_start_nrt_profile"):
            return
        lib.axon_start_nrt_profile.argtypes = [
            ctypes.POINTER(ctypes.c_int64), ctypes.c_size_t]
        lib.axon_start_nrt_profile.restype = ctypes.c_int64
        lib.axon_stop_nrt_profile.argtypes = [ctypes.c_char_p]
        lib.axon_stop_nrt_profile.restype = ctypes.c_int64

        @contextlib.contextmanager
        def _hook(output_dir, device_ids):
            import jax
            jax.devices()
            if device_ids:
                ids = (ctypes.c_int64 * len(device_ids))(*device_ids)
                rc = lib.axon_start_nrt_profile(ids, len(device_ids))
            else:
                rc = lib.axon_start_nrt_profile(None, 0)
            if rc != 0:
                raise RuntimeError(f"axon_start_nrt_profile rc={rc}")
            try:
                yield
            finally:
                n = lib.axon_stop_nrt_profile(str(output_dir).encode())
                print(f"ntff profile: {n} file(s) -> {output_dir}",
                      file=sys.stderr)

        mod = types.ModuleType("antenv.axon_hooks")
        _state = {"hook": _hook}
        mod.set_axon_ntff_profile_hook = lambda h: _state.__setitem__("hook", h)
        mod.get_axon_ntff_profile_hook = lambda: _state["hook"]
        sys.modules["antenv.axon_hooks"] = mod
        import antenv
        antenv.axon_hooks = mod
    except Exception:
        pass


_install_ntff_hook()

F32 = mybir.dt.float32
BF16 = mybir.dt.bfloat16
FP8 = mybir.dt.float8e4
ACT_EXP = mybir.ActivationFunctionType.Exp
ACT_COPY = mybir.ActivationFunctionType.Copy
DR = mybir.MatmulPerfMode.DoubleRow

# Problem constants (hardcoded per contract).
HIDDEN = 1024
HEADS = 16
HEAD_DIM = 64
GAMMA = 0.5
B, S = 2, 2048
N_CORES = 8
GROUPS = N_CORES // B  # head groups per batch
HPC = HEADS // GROUPS  # heads per core
PAIRS = HPC // 2
LAG = 3                # kt software-pipeline lag between pt and ctx matmul
XH_PRESCALE = 64.0     # fp8 prescale; sim psum = PRESCALE^2 * gamma * sim


def emit_kernel(tc, aps, *, S_, C_, QB):
    nc = tc.nc
    CT = C_ // 128          # contraction tiles over hidden
    CC = CT // 2            # fp8 DoubleRow chunks (256 rows each)
    NKT = S_ // 128         # key tiles
    NQB = S_ // QB          # query blocks
    PB = 512                # projection free-block width
    NPB = S_ // PB
    D2 = HPC * HEAD_DIM     # per-core projected channels

    xT_d = aps["xT"]; xh8_d = aps["xh8"]
    wq_d = aps["wq"]; wk_d = aps["wk"]; wv_d = aps["wv"]; wo_d = aps["wo"]
    bq_d = aps["bq"]; bk_d = aps["bk"]; bv_d = aps["bv"]
    out_d = aps["out"]
    mask_d = aps.get("maskmul")

    from contextlib import ExitStack
    stack = ExitStack()

    # --- persistent SBUF tensors ---
    proj = stack.enter_context(tc.tile_pool(name="proj", bufs=1))
    qT_sb = proj.tile([128, PAIRS, S_], BF16)      # head pairs on 64-halves
    kT_sb = proj.tile([128, PAIRS, S_], BF16)
    v2_sb = proj.tile([128, NKT, HPC, HEAD_DIM + 1], BF16)
    xh8_sb = proj.tile([128, CC, 2, S_], FP8)      # DoubleRow interleaved
    wo_sb = proj.tile([128, PAIRS, C_], BF16)
    ctxT2_sb = proj.tile([128, PAIRS, S_], BF16)

    # ---------------- phase 1: load + projections ----------------
    with tc.tile_pool(name="xsp", bufs=1) as xsp, \
         tc.tile_pool(name="wsp", bufs=1) as wsp, \
         tc.tile_pool(name="prjps", bufs=2, space="PSUM") as prjps, \
         tc.tile_pool(name="vps", bufs=2, space="PSUM") as vps:

        # interleave wk/x DMAs per c so the kT accumulation paces with
        # landings; x arrives in column halves
        w_sb = {}
        xs = []
        H = S_ // 2
        for c in range(CT):
            wt = wsp.tile([128, D2], BF16, tag=f"wk{c}", name=f"wk_{c}")
            nc.sync.dma_start(out=wt, in_=wk_d[c * 128:(c + 1) * 128, :])
            w_sb[("wk", c)] = wt
            xt = xsp.tile([128, S_], BF16, tag=f"xs{c}", name=f"xs_{c}")
            nc.sync.dma_start(out=xt[:, 0:H],
                              in_=xT_d[c * 128:(c + 1) * 128, 0:H])
            xs.append(xt)
        for cc in range(CC):
            nc.sync.dma_start(
                out=xh8_sb[:, cc, :, :],
                in_=xh8_d[:, cc, :, :])
        for c in range(CT):
            nc.sync.dma_start(out=xs[c][:, H:S_],
                              in_=xT_d[c * 128:(c + 1) * 128, H:S_])
        for wname, w_d in (("wq", wq_d), ("wv", wv_d)):
            for c in range(CT):
                wt = wsp.tile([128, D2], BF16, tag=f"{wname}{c}",
                              name=f"{wname}_{c}")
                nc.sync.dma_start(out=wt, in_=w_d[c * 128:(c + 1) * 128, :])
                w_sb[(wname, c)] = wt
        nc.sync.dma_start(
            out=wo_sb, in_=wo_d.rearrange("(j p) o -> p j o", p=128))

        b_sb = {}
        for bname, b_d in (("bq", bq_d), ("bk", bk_d), ("bv", bv_d)):
            bt = wsp.tile([128, PAIRS, 1], F32, tag=f"b{bname}")
            nc.sync.dma_start(
                out=bt, in_=b_d.rearrange("(j p) one -> p j one", p=128))
            b_sb[bname] = bt

        # bv broadcast row for the V bias add ([1, D2] varies along free dim)
        bvr = wsp.tile([1, D2], F32, tag="bvr")
        nc.sync.dma_start(out=bvr, in_=bv_d.rearrange("d one -> one d"))
        bvb = wsp.tile([128, D2], F32, tag="bvb")
        nc.gpsimd.partition_broadcast(bvb, bvr, channels=128)

        # q/k projections: W tiles stationary, x moving; accumulate over c
        def emit_proj_nb(wname, bname, dest, nb):
            pss = [prjps.tile([128, PB], F32, tag=f"prj{j}",
                              name=f"prj_{wname}_{nb}_{j}")
                   for j in range(PAIRS)]
            for c in range(CT):
                for j in range(PAIRS):
                    nc.tensor.matmul(
                        pss[j],
                        w_sb[(wname, c)][:, j * 128:(j + 1) * 128],
                        xs[c][:, nb * PB:(nb + 1) * PB],
                        start=(c == 0),
                        stop=(c == CT - 1),
                    )
            for j in range(PAIRS):
                nc.vector.tensor_scalar_add(
                    dest[:, j, nb * PB:(nb + 1) * PB], pss[j],
                    b_sb[bname][:, j, :])

        # V directly in [keys, dh] layout: x tile stationary, WvT moving
        def emit_v_st(st):
            vp = vps.tile([128, D2], F32, tag="vp", name=f"vp_{st}")
            for c in range(CT):
                nc.tensor.matmul(
                    vp,
                    xs[c][:, st * 128:(st + 1) * 128],
                    w_sb[("wv", c)],
                    start=(c == 0),
                    stop=(c == CT - 1),
                )
            nc.vector.tensor_add(
                v2_sb[:, st, :, 0:HEAD_DIM],
                vp.rearrange("p (h d) -> p h d", h=HPC),
                bvb.rearrange("p (h d) -> p h d", h=HPC),
            )

        # consume x half 0 first (its DMA lands first), then half 1
        emit_proj_nb("wk", "bk", kT_sb, 0)
        emit_proj_nb("wq", "bq", qT_sb, 0)
        for st in range(4):
            emit_v_st(st)
        emit_proj_nb("wk", "bk", kT_sb, 1)
        emit_proj_nb("wq", "bq", qT_sb, 1)
        for st in range(4, 8):
            emit_v_st(st)
        for nb in range(2, NPB):
            emit_proj_nb("wk", "bk", kT_sb, nb)
            emit_proj_nb("wq", "bq", qT_sb, nb)
        for st in range(8, NKT):
            emit_v_st(st)
        nc.vector.memset(v2_sb[:, :, :, HEAD_DIM:HEAD_DIM + 1], 1.0)

    # ---------------- phase 2: attention main loop ----------------
    epool = stack.enter_context(tc.tile_pool(name="epool", bufs=2))
    espool = stack.enter_context(tc.tile_pool(name="espool", bufs=2))
    ptpool = stack.enter_context(tc.tile_pool(name="ptpool", bufs=4))
    smallpool = stack.enter_context(tc.tile_pool(name="smallpool", bufs=2))
    outstg = stack.enter_context(tc.tile_pool(name="outstg", bufs=3))
    mpool = (stack.enter_context(tc.tile_pool(name="mpool", bufs=2))
             if mask_d is not None else None)

    with tc.tile_pool(name="mmps", bufs=1, space="PSUM") as mmps, \
         tc.tile_pool(name="ctxps", bufs=1, space="PSUM") as ctxps:

        mmctr = [0]

        def mm_tile(name):
            t = mmps.tile([128, 2, QB], F32, tag=f"T{mmctr[0] % 3}",
                          name=name)
            mmctr[0] += 1
            return t

        def emit_ctx_pair(ctx_pair, j, kt, pt):
            for hi in range(2):
                nc.tensor.matmul(
                    ctx_pair[hi],
                    v2_sb[:, kt, 2 * j + hi, :],
                    pt[:, hi, :],
                    start=(kt == 0),
                    stop=(kt == NKT - 1),
                    skip_group_check=True,
                )

        def emit_out_qt(qt):
            # one query tile's partial out-projection: [128, 1024]
            op = mm_tile(f"op_{qt}")
            for ob in range(2):
                for j in range(PAIRS):
                    nc.tensor.matmul(
                        op[:, ob, :],
                        ctxT2_sb[:, j, qt * 128:(qt + 1) * 128],
                        wo_sb[:, j, ob * QB:(ob + 1) * QB],
                        start=(j == 0),
                        stop=(j == PAIRS - 1),
                    )
            ostg = outstg.tile([128, 2, QB], F32, tag="ostg",
                               name=f"ostg_{qt}")
            nc.vector.tensor_copy(ostg[:, 0, :], op[:, 0, :])
            nc.scalar.activation(out=ostg[:, 1, :], in_=op[:, 1, :],
                                 func=ACT_COPY)
            for ob in range(2):
                nc.sync.dma_start(
                    out=out_d[qt * 128:(qt + 1) * 128,
                              ob * QB:(ob + 1) * QB],
                    in_=ostg[:, ob, :],
                )

        def emit_divisions_pair(qb, j, ctx_pair):
            qsl = slice(qb * QB, (qb + 1) * QB)
            for hi in range(2):
                s0 = smallpool.tile([1, QB], F32, tag="s0",
                                    name=f"s0_{qb}_{j}_{hi}")
                nc.vector.tensor_copy(
                    s0, ctx_pair[hi][HEAD_DIM:HEAD_DIM + 1, :])
                r0 = smallpool.tile([1, QB], F32, tag="r0",
                                    name=f"r0_{qb}_{j}_{hi}")
                nc.vector.reciprocal_approx_fast(r0, s0)
                rb = smallpool.tile([HEAD_DIM, QB], F32, tag="rb",
                                    name=f"rb_{qb}_{j}_{hi}")
                nc.gpsimd.partition_broadcast(rb, r0, channels=HEAD_DIM)
                nc.vector.tensor_mul(
                    ctxT2_sb[hi * 64:hi * 64 + 64, j, qsl],
                    ctx_pair[hi][0:HEAD_DIM, :],
                    rb,
                )

        carry = None        # (qb, j, ctx_pair, undrained pops) of prev pass
        pending_out = None  # qb whose out-projection is ready to emit
        inv_ps2 = -1.0 / (XH_PRESCALE * XH_PRESCALE)
        for qb in range(NQB):
            qsl = slice(qb * QB, (qb + 1) * QB)
            e_tiles = {}
            for j in range(PAIRS):
                ctx_pair = [
                    ctxps.tile([HEAD_DIM + 1, QB], F32, tag=f"cx{hi}",
                               name=f"ctx_{qb}_{j}_{hi}")
                    for hi in range(2)
                ]
                pending = []
                for kt in range(NKT):
                    ksl = slice(kt * 128, (kt + 1) * 128)
                    if j == 0 and kt % 2 == 0:
                        # sim for kt, kt+1 (fp8 DoubleRow) and shared E
                        spt = mm_tile(f"sim_{qb}_{kt}")
                        for par in range(2):
                            kk = slice((kt + par) * 128, (kt + par + 1) * 128)
                            for cc in range(CC):
                                nc.tensor.matmul(
                                    spt[:, par, :],
                                    xh8_sb[:, cc, :, kk],
                                    xh8_sb[:, cc, :, qsl],
                                    start=(cc == 0),
                                    stop=(cc == CC - 1),
                                    perf_mode=DR,
                                )
                        e2 = epool.tile([128, 2, QB], BF16, tag=f"E{kt // 2}",
                                        name=f"E_{qb}_{kt}")
                        nc.scalar.activation(out=e2, in_=spt, func=ACT_EXP,
                                             scale=inv_ps2)
                        if mask_d is not None:
                            for par in range(2):
                                kk = slice((kt + par) * 128,
                                           (kt + par + 1) * 128)
                                m_sb = mpool.tile([128, QB], BF16, tag="msk")
                                nc.sync.dma_start(out=m_sb,
                                                  in_=mask_d[kk, qsl])
                                nc.vector.tensor_mul(
                                    e2[:, par, :], e2[:, par, :], m_sb)
                        e_tiles[kt // 2] = e2
                    eb = (e_tiles[kt // 2][:, kt % 2, :]
                          .unsqueeze(1).to_broadcast([128, 2, QB]))

                    sc_t = mm_tile(f"sc_{qb}_{kt}_{j}")
                    for hi in range(2):
                        pr = slice(hi * 64, hi * 64 + 64)
                        nc.tensor.matmul(
                            sc_t[:, hi, :],
                            kT_sb[pr, j, ksl],
                            qT_sb[pr, j, qsl],
                            start=True,
                            stop=True,
                        )
                    es_t = espool.tile([128, 2, QB], BF16, tag=f"es{j}",
                                       name=f"es_{qb}_{kt}_{j}")
                    nc.scalar.activation(out=es_t, in_=sc_t, func=ACT_EXP)
                    pt = ptpool.tile([128, 2, QB], BF16, tag=f"pt{j}",
                                     name=f"pt_{qb}_{kt}_{j}")
                    nc.vector.tensor_mul(pt, es_t, eb)
                    pending.append((kt, pt))

                    # drain the previous pass's pops, then its divisions
                    if carry is not None and kt <= 2:
                        cqb, cj, cctx, cpend = carry
                        k0, p0 = cpend.pop(0)
                        emit_ctx_pair(cctx, cj, k0, p0)
                        if not cpend:
                            emit_divisions_pair(cqb, cj, cctx)
                            if cj == PAIRS - 1:
                                pending_out = cqb
                            carry = None
                    if len(pending) > LAG:
                        k0, p0 = pending.pop(0)
                        emit_ctx_pair(ctx_pair, j, k0, p0)
                    # out-projection of the qb finished two passes ago
                    if j == 1 and 2 <= kt <= 5 and pending_out is not None:
                        emit_out_qt(pending_out * (QB // 128) + kt - 2)
                        if kt == 5:
                            pending_out = None
                carry = (qb, j, ctx_pair, pending)
        cqb, cj, cctx, cpend = carry
        for k0, p0 in cpend:
            emit_ctx_pair(cctx, cj, k0, p0)
        emit_divisions_pair(cqb, cj, cctx)
        for qt in range(cqb * (QB // 128), (cqb + 1) * (QB // 128)):
            emit_out_qt(qt)

    stack.close()


def build_nc(*, S_=S, C_=HIDDEN, QB=512, with_mask=False,
             enable_asserts=False):
    nc = bacc.Bacc(
        "TRN2", target_bir_lowering=False, debug=False,
        enable_asserts=enable_asserts,
    )
    D2 = HPC * HEAD_DIM
    aps = {}
    aps["xT"] = nc.dram_tensor("xT", [C_, S_], BF16, kind="ExternalInput").ap()
    aps["xh8"] = nc.dram_tensor(
        "xh8", [128, C_ // 256, 2, S_], FP8, kind="ExternalInput").ap()
    for n in ("wq", "wk", "wv"):
        aps[n] = nc.dram_tensor(n, [C_, D2], BF16, kind="ExternalInput").ap()
    aps["wo"] = nc.dram_tensor("wo", [D2, C_], BF16, kind="ExternalInput").ap()
    for n in ("bq", "bk", "bv"):
        aps[n] = nc.dram_tensor(n, [D2, 1], F32, kind="ExternalInput").ap()
    if with_mask:
        aps["maskmul"] = nc.dram_tensor(
            "maskmul", [S_, S_], BF16, kind="ExternalInput").ap()
    aps["out"] = nc.dram_tensor("out", [S_, C_], F32, kind="ExternalOutput").ap()

    with tile.TileContext(nc) as tc:
        emit_kernel(tc, aps, S_=S_, C_=C_, QB=QB)
    nc.compile()
    return nc


def host_prepare(x, attn_mask, Wq, bq, Wk, bk, Wv, bv, Wo, bo, *,
                 S_=S, C_=HIDDEN, n_cores=N_CORES):
    """Build the per-core input maps. Returns (in_maps, with_mask)."""
    import ml_dtypes
    BF = ml_dtypes.bfloat16
    x = np.asarray(x, np.float32)
    B_ = x.shape[0]
    groups = n_cores // B_
    Wq = np.asarray(Wq, np.float32); Wk = np.asarray(Wk, np.float32)
    Wv = np.asarray(Wv, np.float32); Wo = np.asarray(Wo, np.float32)
    bq = np.asarray(bq, np.float32); bk = np.asarray(bk, np.float32)
    bv = np.asarray(bv, np.float32)

    inv_sqrt_d = 1.0 / math.sqrt(HEAD_DIM)
    WqT = np.ascontiguousarray((Wq * inv_sqrt_d).T).astype(BF)
    WkT = np.ascontiguousarray(Wk.T).astype(BF)
    WvT = np.ascontiguousarray(Wv.T).astype(BF)
    WoT = np.ascontiguousarray(Wo.T).astype(BF)      # [C(c), C(o)]
    bq = bq * inv_sqrt_d

    mask = np.asarray(attn_mask)
    with_mask = bool(mask.any())
    maskmul = None
    if with_mask:
        # reference: where(mask, -inf) -> multiplicative 0/1 on exp values
        maskmul = np.where(mask.T, 0.0, 1.0).astype(BF)
        maskmul = np.ascontiguousarray(maskmul)

    in_maps = []
    for core in range(n_cores):
        b, g = divmod(core, groups)
        xb = x[b]                                   # [S, C]
        xT = np.ascontiguousarray(xb.T).astype(BF)  # [C, S]
        norms = np.linalg.norm(xb, axis=1)          # [S]
        scale = (XH_PRESCALE * math.sqrt(GAMMA)
                 / np.maximum(norms, 1e-12)).astype(np.float32)
        xh = (xb.T * scale[None, :]).astype(ml_dtypes.float8_e4m3)
        # DoubleRow interleave: xh8[p, cc, i, s] = xh[cc*256 + i*128 + p, s]
        xh8 = np.ascontiguousarray(
            xh.reshape(C_ // 256, 2, 128, S_).transpose(2, 0, 1, 3))
        D2 = HPC * HEAD_DIM
        ch = slice(g * D2, (g + 1) * D2)
        m = {
            "xT": xT,
            "xh8": xh8,
            "wq": np.ascontiguousarray(WqT[:, ch]),
            "wk": np.ascontiguousarray(WkT[:, ch]),
            "wv": np.ascontiguousarray(WvT[:, ch]),
            "wo": np.ascontiguousarray(WoT[ch, :]),
            "bq": np.ascontiguousarray(bq[ch]).reshape(-1, 1),
            "bk": np.ascontiguousarray(bk[ch]).reshape(-1, 1),
            "bv": np.ascontiguousarray(bv[ch]).reshape(-1, 1),
        }
        if with_mask:
            m["maskmul"] = maskmul
        in_maps.append(m)
    return in_maps, with_mask


_NC_CACHE = {}


def _get_nc(with_mask):
    key = with_mask
    if key not in _NC_CACHE:
        _NC_CACHE[key] = build_nc(with_mask=with_mask)
    return _NC_CACHE[key]


LAST_RESULTS = None


def kernel(**inputs):
    global LAST_RESULTS
    in_maps, with_mask = host_prepare(
        inputs["x"], inputs["attn_mask"],
        inputs["Wq"], inputs["bq"], inputs["Wk"], inputs["bk"],
        inputs["Wv"], inputs["bv"], inputs["Wo"], inputs["bo"],
    )
    nc = _get_nc(with_mask)
    res = run_bass_kernel_spmd(nc, in_maps, core_ids=list(range(N_CORES)))
    LAST_RESULTS = res
    bo = np.asarray(inputs["bo"], np.float32)
    out = np.zeros((B, S, HIDDEN), np.float32)
    groups = N_CORES // B
    for core in range(N_CORES):
        b = core // groups
        out[b] += np.asarray(res.results[core]["out"], np.float32)
    out += bo[None, None, :]
    return out
